# revision 1
# baseline (speedup 1.0000x reference)
"""CentroidInstanceLoss on 8 Trainium2 NeuronCores (Bass/Tile).

Data-parallel over points: each of the 8 cores processes N/8 = 32768 points.
Per-core segment sums (via one-hot matmuls) are combined with a
ReduceScatter; the [512, 257] centroid(+pull-weight) table is AllGathered
back; a second pass over the points computes the pull term; the push term
uses partition-rotated centroid diffs on the core owning each subbatch.
Host does only O(S*L) label bookkeeping and the final ~70-float combine.
"""

import numpy as np

import concourse.bass as bass
import concourse.bacc as bacc
import concourse.mybir as mybir
import concourse.tile as tile

f32 = mybir.dt.float32
f16 = mybir.dt.float16
HALF = True
fdat = f16 if HALF else f32

# Problem shape (hardcoded per contract).
N_TOTAL = 262144
D = 256
S = 8
L = 64
NSEG = S * L  # 512
NCORES = 8
DELTA_V = 0.5
DELTA_D = 1.5

AluOp = mybir.AluOpType
ActFn = mybir.ActivationFunctionType


def build_nc(n_core: int, use_collectives: bool = True, reps: int = 1,
             phases: tuple = ("p1", "cc", "push", "p2")):
    """Build the SPMD Bass program for one core holding n_core points.

    use_collectives=False builds a single-core variant (collectives replaced
    with local DMA) for TimelineSim profiling. reps>1 replicates the body for
    marginal-time measurement on hardware.
    """
    assert n_core % 128 == 0
    T = n_core // 128  # point tiles per core
    G = min(8, T)      # norm-batch group size
    assert T % G == 0

    nc = bacc.Bacc(
        "TRN2", target_bir_lowering=False, debug=False,
        num_devices=NCORES if use_collectives else 1,
    )

    x_in = nc.dram_tensor("x", [n_core, D], fdat, kind="ExternalInput")
    segrow_in = nc.dram_tensor("segrow", [n_core], fdat, kind="ExternalInput")
    segcol_in = nc.dram_tensor("segcol", [128, T], f32, kind="ExternalInput")
    sbcol_in = nc.dram_tensor("sbcol", [128, T], f32, kind="ExternalInput")
    iota512_in = nc.dram_tensor("iota512", [128, NSEG], fdat, kind="ExternalInput")
    iotapc_in = nc.dram_tensor("iotapc", [128, 4], f32, kind="ExternalInput")
    iota8_in = nc.dram_tensor("iota8", [128, S], f32, kind="ExternalInput")
    ones_in = nc.dram_tensor("ones1", [1, 128], fdat, kind="ExternalInput")
    perms_in = nc.dram_tensor("perms", [L, L - 1, L], fdat, kind="ExternalInput")
    wblk_in = nc.dram_tensor("wblk", [L, 1], f32, kind="ExternalInput")
    crecip_in = nc.dram_tensor("crecip", [L, 1], f32, kind="ExternalInput")

    lpull_out = nc.dram_tensor("lpull", [S, 1], f32, kind="ExternalOutput")
    qrot_out = nc.dram_tensor("qrot", [L, L], f32, kind="ExternalOutput")

    segrow_v = segrow_in.ap().rearrange("(t i) -> t i", i=128)  # [T, 128]

    with tile.TileContext(nc) as tc:
        with (
            tc.tile_pool(name="const", bufs=1) as constp,
            tc.tile_pool(name="norm", bufs=1) as normp,
            tc.tile_pool(name="mu", bufs=1) as mup,
            tc.tile_pool(name="dram", bufs=1, space="DRAM") as dram,
            tc.tile_pool(name="x1", bufs=4) as xp1,
            tc.tile_pool(name="oh", bufs=4) as ohp,
            tc.tile_pool(name="sqc", bufs=2) as sqcp,
        ):
            # ---- constants ----
            iota512_sb = constp.tile([128, NSEG], fdat)
            nc.sync.dma_start(iota512_sb[:], iota512_in[:])
            iotapc_sb = constp.tile([128, 4], f32)
            nc.sync.dma_start(iotapc_sb[:], iotapc_in[:])
            iota8_sb = constp.tile([128, S], f32)
            nc.sync.dma_start(iota8_sb[:], iota8_in[:])
            ones_sb = constp.tile([1, 128], fdat)
            nc.sync.dma_start(ones_sb[:], ones_in[:])
            segcol_sb = constp.tile([128, T], f32)
            nc.sync.dma_start(segcol_sb[:], segcol_in[:])
            sbcol_sb = constp.tile([128, T], f32)
            nc.sync.dma_start(sbcol_sb[:], sbcol_in[:])
            wblk_sb = constp.tile([L, 1], f32)
            nc.sync.dma_start(wblk_sb[:], wblk_in[:])
            crecip_sb = constp.tile([L, 1], f32)
            nc.sync.dma_start(crecip_sb[:], crecip_in[:])
            perms_sb = constp.tile([L, L - 1, L], fdat)
            nc.sync.dma_start(perms_sb[:], perms_in[:])
            negdv_sb = constp.tile([128, 1], f32)
            nc.vector.memset(negdv_sb[:], -DELTA_V)

            for rep in range(reps):
                ss_all = normp.tile([128, T], f32, tag="ss", name="ss_all")
                rr_all = normp.tile([128, T], f32, tag="rr", name="rr_all")

                # ---- pass 1: per-core segment sums of normalized points ----
                with tc.tile_pool(name="psum1", bufs=1, space="PSUM") as psum1:
                    ps_sums = [
                        psum1.tile([128, D], f32, tag=f"sums{c}", name=f"ps_sums{c}")
                        for c in range(4)
                    ]
                    for g in range(T // G if "p1" in phases else 0):
                        t0 = g * G
                        xb = xp1.tile([128, G, D], fdat, tag="x1t")
                        nc.sync.dma_start(
                            xb[:],
                            x_in[t0 * 128:(t0 + G) * 128, :].rearrange(
                                "(g p) d -> p g d", p=128),
                        )
                        for j in range(G):
                            t = t0 + j
                            sink = sqcp.tile([128, D], fdat, tag="sq_sink")
                            if j % 2 == 0:
                                nc.vector.scalar_tensor_tensor(
                                    sink[:], xb[:, j, :], 1.0, xb[:, j, :],
                                    op0=AluOp.bypass, op1=AluOp.mult,
                                    accum_out=ss_all[:, t:t + 1],
                                )
                            else:
                                nc.scalar.activation(
                                    sink[:], xb[:, j, :], ActFn.Square,
                                    accum_out=ss_all[:, t:t + 1],
                                )
                        sqc = sqcp.tile([128, G], f32, tag="sqc")
                        nc.scalar.activation(
                            sqc[:], ss_all[:, g * G:(g + 1) * G], ActFn.Sqrt
                        )
                        nc.vector.tensor_scalar_add(sqc[:], sqc[:], 1e-8)
                        nc.vector.reciprocal(rr_all[:, g * G:(g + 1) * G], sqc[:])
                        for j in range(G):
                            t = t0 + j
                            oh = ohp.tile([128, NSEG], fdat, tag="oh")
                            nc.gpsimd.tensor_scalar(
                                oh[:, 0:384], iota512_sb[:, 0:384],
                                segcol_sb[:, t:t + 1], rr_all[:, t:t + 1],
                                op0=AluOp.is_equal, op1=AluOp.mult,
                            )
                            nc.vector.tensor_scalar(
                                oh[:, 384:NSEG], iota512_sb[:, 384:NSEG],
                                segcol_sb[:, t:t + 1], rr_all[:, t:t + 1],
                                op0=AluOp.is_equal, op1=AluOp.mult,
                            )
                            for c in range(4):
                                nc.tensor.matmul(
                                    ps_sums[c][:],
                                    oh[:, c * 128:(c + 1) * 128],
                                    xb[:, j, :],
                                    start=(t == 0), stop=(t == T - 1),
                                )

                    rs_in = dram.tile([NSEG, D], f32, tag="rs_in", name="rs_in")
                    for c in range(4):
                        sums_sb = sqcp.tile(
                            [128, D], f32, tag="sums_sb", name="sums_sb"
                        )
                        nc.vector.tensor_copy(sums_sb[:], ps_sums[c][:])
                        nc.sync.dma_start(
                            rs_in[c * 128:(c + 1) * 128, :], sums_sb[:]
                        )

                # ---- combine centroid table across cores ----
                rs_out = dram.tile([L, D], f32, tag="rs_out", name="rs_out")
                if "cc" not in phases:
                    nc.sync.dma_start(rs_out[:], rs_in[0:L, :])
                elif use_collectives:
                    nc.gpsimd.collective_compute(
                        "ReduceScatter", AluOp.add,
                        replica_groups=[list(range(NCORES))],
                        ins=[rs_in.opt()], outs=[rs_out.opt()],
                    )
                else:
                    nc.sync.dma_start(rs_out[:], rs_in[0:L, :])
                musb_raw = mup.tile([L, D], f32, tag="musb", name="musb_raw")
                nc.sync.dma_start(musb_raw[:], rs_out[:])
                muaug = mup.tile([L, D + 1], f32, tag="muaug", name="muaug")
                nc.vector.tensor_scalar(
                    muaug[:, 0:D], musb_raw[:], crecip_sb[:, 0:1], None,
                    op0=AluOp.mult,
                )
                nc.vector.tensor_copy(muaug[:, D:D + 1], wblk_sb[:])
                ag_in = dram.tile([L, D + 1], f32, tag="ag_in", name="ag_in")
                nc.sync.dma_start(ag_in[:], muaug[:])
                ag_out = dram.tile(
                    [NSEG, D + 1], f32, tag="ag_out", name="ag_out",
                    addr_space="Shared" if use_collectives else "Local",
                )
                if use_collectives and "cc" in phases:
                    nc.gpsimd.collective_compute(
                        "AllGather", AluOp.bypass,
                        replica_groups=[list(range(NCORES))],
                        ins=[ag_in.opt()], outs=[ag_out.opt()],
                    )
                else:
                    for c in range(S):
                        nc.sync.dma_start(
                            ag_out[c * L:(c + 1) * L, :], ag_in[:]
                        )
                mut_sb = mup.tile([128, 4, D + 1], f32, tag="mut", name="mut_sb")
                nc.sync.dma_start(
                    mut_sb[:], ag_out.rearrange("(c p) d -> p c d", p=128)
                )
                mut_h = mup.tile([128, 4, D + 1], fdat, tag="muth", name="mut_h")
                nc.vector.tensor_copy(mut_h[:], mut_sb[:])

                # ---- push: pairwise centroid L1 distances (own subbatch) ----
                q_sb = mup.tile([L, L], f32, tag="q", name="q_sb")
                nc.vector.memset(q_sb[:, 0:1], 0.0)
                mua_h = mup.tile([L, D], fdat, tag="muah", name="mua_h")
                nc.vector.tensor_copy(mua_h[:], muaug[:, 0:D])
                with (
                    tc.tile_pool(name="rotps", bufs=2, space="PSUM") as rotpsp,
                    tc.tile_pool(name="pdiff", bufs=3) as pdp,
                ):
                    for k in range(1, L if "push" in phases else 1):
                        ps_rot = rotpsp.tile([L, D], f32, tag="rotps")
                        nc.tensor.matmul(
                            ps_rot[:], perms_sb[:, k - 1, :], mua_h[:],
                            start=True, stop=True,
                        )
                        pdiff = pdp.tile([L, D], f32, tag="pdiff")
                        nc.vector.tensor_sub(pdiff[:], mua_h[:], ps_rot[:])
                        psink = pdp.tile([L, D], f32, tag="psink")
                        nc.scalar.activation(
                            psink[:], pdiff[:], ActFn.Abs,
                            accum_out=q_sb[:, k:k + 1],
                        )
                nc.sync.dma_start(qrot_out[:], q_sb[:])

                # ---- pass 2: pull term ----
                with (
                    tc.tile_pool(name="x2", bufs=6) as xp2,
                    tc.tile_pool(name="srow", bufs=4) as srowp,
                    tc.tile_pool(name="bcps", bufs=3, space="PSUM") as bcpsp,
                    tc.tile_pool(name="bcsb", bufs=3) as bcsbp,
                    tc.tile_pool(name="oht", bufs=4) as ohtp,
                    tc.tile_pool(name="mups", bufs=3, space="PSUM") as mupsp,
                    tc.tile_pool(name="pullps", bufs=1, space="PSUM") as pullpsp,
                    tc.tile_pool(name="diff", bufs=3) as diffp,
                    tc.tile_pool(name="sink2", bufs=2) as sink2p,
                    tc.tile_pool(name="small", bufs=4) as smallp,
                ):
                    ps_pull = pullpsp.tile([S, 1], f32, tag="pull", name="ps_pull")
                    if "p2" not in phases:
                        nc.vector.memset(ps_pull[:], 0.0)
                    for t in range(T if "p2" in phases else 0):
                        j = t % G
                        if j == 0:
                            xb2 = xp2.tile([128, G, D], fdat, tag="x2t")
                            nc.sync.dma_start(
                                xb2[:],
                                x_in[t * 128:(t + G) * 128, :].rearrange(
                                    "(g p) d -> p g d", p=128),
                            )
                            srow8 = srowp.tile([1, G * 128], fdat, tag="srow")
                            nc.sync.dma_start(
                                srow8[:],
                                segrow_in.ap()[t * 128:(t + G) * 128]
                                .rearrange("(a i) -> a i", a=1),
                            )
                        xt = xb2[:, j, :]
                        srow = srow8[:, j * 128:(j + 1) * 128]
                        ps_bc = bcpsp.tile([128, 128], f32, tag="bc")
                        nc.tensor.matmul(
                            ps_bc[:], ones_sb[:], srow[:], start=True, stop=True
                        )
                        bc_sb = bcsbp.tile([128, 128], fdat, tag="bcsb")
                        nc.vector.tensor_copy(bc_sb[:], ps_bc[:])
                        oht = ohtp.tile([128, NSEG], fdat, tag="oht")
                        for c in range(4):
                            nc.gpsimd.tensor_scalar(
                                oht[:, c * 128:(c + 1) * 128], bc_sb[:],
                                iotapc_sb[:, c:c + 1], None,
                                op0=AluOp.is_equal,
                            )
                        ps_mu = mupsp.tile([128, D + 1], f32, tag="mu")
                        for c in range(4):
                            nc.tensor.matmul(
                                ps_mu[:],
                                oht[:, c * 128:(c + 1) * 128],
                                mut_h[:, c, :],
                                start=(c == 0), stop=(c == 3),
                            )
                        diff = diffp.tile([128, D], f32, tag="diff")
                        nc.vector.scalar_tensor_tensor(
                            diff[:], xt, rr_all[:, t:t + 1], ps_mu[:, 0:D],
                            op0=AluOp.mult, op1=AluOp.subtract,
                        )
                        sink = sink2p.tile([128, D], f32, tag="sink2")
                        d1 = smallp.tile([128, 1], f32, tag="d1")
                        nc.scalar.activation(
                            sink[:], diff[:], ActFn.Abs, accum_out=d1[:]
                        )
                        t1 = smallp.tile([128, 1], f32, tag="t1")
                        nc.scalar.activation(
                            t1[:], d1[:], ActFn.Relu, bias=negdv_sb[:]
                        )
                        t2 = smallp.tile([128, 1], f32, tag="t2")
                        nc.vector.tensor_mul(t2[:], t1[:], t1[:])
                        v = smallp.tile([128, 1], f32, tag="v")
                        nc.vector.tensor_mul(v[:], t2[:], ps_mu[:, D:D + 1])
                        ohsb = smallp.tile([128, S], f32, tag="ohsb")
                        nc.vector.tensor_scalar(
                            ohsb[:], iota8_sb[:], sbcol_sb[:, t:t + 1], None,
                            op0=AluOp.is_equal,
                        )
                        nc.tensor.matmul(
                            ps_pull[:], ohsb[:], v[:],
                            start=(t == 0), stop=(t == T - 1),
                        )
                    lpull_sb = smallp.tile([S, 1], f32, tag="lpull_sb")
                    nc.vector.tensor_copy(lpull_sb[:], ps_pull[:])
                    nc.sync.dma_start(lpull_out[:], lpull_sb[:])

    nc.compile()
    return nc


def host_tables(labels: np.ndarray, subbatch: np.ndarray):
    """Everything derivable from the integer inputs alone."""
    seg = (subbatch.astype(np.int64) * L + labels.astype(np.int64)).astype(np.int32)
    counts = np.bincount(seg, minlength=NSEG).astype(np.float64)  # [512]
    present = counts > 0
    M = present.reshape(S, L).sum(axis=1).astype(np.float64)  # [S]
    valid = M > 1.0
    # per-seg pull weight: valid(sb)/(M_sb * count_s); 0 for invalid sb
    M_per_seg = np.repeat(M, L)
    valid_per_seg = np.repeat(valid, L)
    w = np.where(
        valid_per_seg, 1.0 / (M_per_seg * np.maximum(counts, 1.0)), 0.0
    ).astype(np.float32)
    crecip = (1.0 / np.maximum(counts, 1.0)).astype(np.float32)
    return seg, counts, present, M, valid, w, crecip


def make_in_maps(outputs: np.ndarray, labels: np.ndarray, subbatch: np.ndarray):
    n = outputs.shape[0]
    n_core = n // NCORES
    T = n_core // 128
    seg, counts, present, M, valid, w, crecip = host_tables(labels, subbatch)
    segf = seg.astype(np.float32)
    sbf = subbatch.astype(np.float32)

    iota512 = np.broadcast_to(
        np.arange(NSEG, dtype=np.float32), (128, NSEG)
    ).copy()
    iotapc = (
        np.arange(4, dtype=np.float32)[None, :] * 128.0
        + np.arange(128, dtype=np.float32)[:, None]
    ).copy()  # [128, 4]
    iota8 = np.broadcast_to(np.arange(S, dtype=np.float32), (128, S)).copy()
    ones1 = np.ones((1, 128), dtype=np.float32)
    pp, kk, mm = np.meshgrid(
        np.arange(L), np.arange(1, L), np.arange(L), indexing="ij")
    perms = (pp == (mm + kk) % L).astype(np.float32)  # [L, L-1, L]

    in_maps = []
    for c in range(NCORES):
        sl = slice(c * n_core, (c + 1) * n_core)
        segc = segf[sl]
        sbc = sbf[sl]
        blk = slice(c * L, (c + 1) * L)
        in_maps.append({
            "x": np.ascontiguousarray(outputs[sl]).astype(np.float16) if HALF else np.ascontiguousarray(outputs[sl]),
            "segrow": segc.astype(np.float16) if HALF else segc,
            "segcol": np.ascontiguousarray(segc.reshape(T, 128).T),
            "sbcol": np.ascontiguousarray(sbc.reshape(T, 128).T),
            "iota512": iota512.astype(np.float16) if HALF else iota512,
            "iotapc": iotapc,
            "iota8": iota8,
            "ones1": ones1.astype(np.float16) if HALF else ones1,
            "perms": perms.astype(np.float16) if HALF else perms,
            "wblk": w[blk].reshape(L, 1),
            "crecip": crecip[blk].reshape(L, 1),
        })
    return in_maps, (seg, counts, present, M, valid, w, crecip)


def combine(results, tables, n: int):
    """Host combine of the per-core outputs into the scalar loss."""
    seg, counts, present, M, valid, w, crecip = tables
    pull_total = np.float64(0.0)
    for r in results:
        pull_total += r["lpull"].astype(np.float64).sum()

    push_total = np.float64(0.0)
    pres_sl = present.reshape(S, L)
    for sb in range(S):
        if not valid[sb]:
            continue
        q = results[sb]["qrot"].astype(np.float64)  # [64(a), 64(k)]
        a = np.arange(L)
        dist = np.zeros((L, L))
        for k in range(1, L):
            dist[a, (a + k) % L] = q[:, k]
        p = pres_sl[sb]
        mask = p[:, None] & p[None, :] & ~np.eye(L, dtype=bool)
        r = np.maximum(2.0 * DELTA_D - dist, 0.0) ** 2
        push = np.where(mask, r, 0.0).sum()
        push_total += push / max(M[sb] * (M[sb] - 1.0), 1.0)

    return np.float32((pull_total + push_total) / n)


_NC_CACHE: dict = {}


def _get_nc(n_core: int):
    if n_core not in _NC_CACHE:
        _NC_CACHE[n_core] = build_nc(n_core)
    return _NC_CACHE[n_core]


def kernel(outputs, labels, subbatch_indices):
    from concourse.bass_utils import run_bass_kernel_spmd

    outputs = np.asarray(outputs, dtype=np.float32)
    labels = np.asarray(labels, dtype=np.int32)
    subbatch_indices = np.asarray(subbatch_indices, dtype=np.int32)
    n = outputs.shape[0]
    n_core = n // NCORES

    nc = _get_nc(n_core)
    in_maps, tables = make_in_maps(outputs, labels, subbatch_indices)
    res = run_bass_kernel_spmd(nc, in_maps, list(range(NCORES)))
    return combine(res.results, tables, n)



# revision 12
# speedup vs baseline: 1.5913x; 1.5913x over previous
"""CentroidInstanceLoss on 8 Trainium2 NeuronCores (Bass/Tile).

Data-parallel over points: each of the 8 cores processes N/8 = 32768 points.
Host precomputes 0/1 one-hot encodings of the integer (subbatch,label) ids
(point-major for the segment-sum pass, segment-major for the centroid gather
pass) so the device does no per-point compare ops at all. Per-core segment
sums (one-hot matmuls against SBUF-resident normalized points) are combined
with a single AllReduce of the [512, 256] sum table; each core then forms
the full centroid table locally, computes the pull term for its points and
the push term for its own subbatch, and emits 9 floats. Host sums 72 floats.
"""

import numpy as np

import concourse.bass as bass
import concourse.bacc as bacc
import concourse.mybir as mybir
import concourse.tile as tile

f32 = mybir.dt.float32
f16 = mybir.dt.float16

# Problem shape (hardcoded per contract).
N_TOTAL = 262144
D = 256
S = 8
L = 64
NSEG = S * L  # 512
NCORES = 8
DELTA_V = 0.5
DELTA_D = 1.5

AluOp = mybir.AluOpType
ActFn = mybir.ActivationFunctionType


def build_nc(n_core: int, use_collectives: bool = True, reps: int = 1):
    """Build the SPMD Bass program for one core holding n_core points."""
    assert n_core % 128 == 0
    T = n_core // 128  # point tiles per core
    G = min(8, T)      # group size for batched DMA
    assert T % G == 0

    nc = bacc.Bacc(
        "TRN2", target_bir_lowering=False, debug=False,
        num_devices=NCORES if use_collectives else 1,
    )

    x_in = nc.dram_tensor("x", [n_core, D], f16, kind="ExternalInput")
    oh_in = nc.dram_tensor("oh", [n_core, NSEG], f16, kind="ExternalInput")
    oht_in = nc.dram_tensor("oht", [n_core, NSEG], f16, kind="ExternalInput")
    ohsb_in = nc.dram_tensor("ohsb", [n_core, S], f16, kind="ExternalInput")
    crecip_in = nc.dram_tensor("crecip", [128, 4], f32, kind="ExternalInput")
    wpc_in = nc.dram_tensor("wpc", [128, 4], f32, kind="ExternalInput")
    ownsel_in = nc.dram_tensor("ownsel", [128, 4 * L], f16, kind="ExternalInput")
    perms_in = nc.dram_tensor("perms", [L, (L - 1) * L], f16, kind="ExternalInput")
    pmk_in = nc.dram_tensor("pmk", [L, L - 1], f32, kind="ExternalInput")
    ones64_in = nc.dram_tensor("ones64", [L, 1], f32, kind="ExternalInput")

    lpull_out = nc.dram_tensor("lpull", [S, 1], f32, kind="ExternalOutput")
    push_out = nc.dram_tensor("push", [1, 1], f32, kind="ExternalOutput")

    with tile.TileContext(nc) as tc:
        with (
            tc.tile_pool(name="const", bufs=1) as constp,
            tc.tile_pool(name="xs", bufs=1) as xsp,
            tc.tile_pool(name="norm", bufs=1) as normp,
            tc.tile_pool(name="mu", bufs=1) as mup,
            tc.tile_pool(name="dram", bufs=1, space="DRAM") as dram,
        ):
            # ---- constants ----
            crecip_sb = constp.tile([128, 4], f32)
            nc.sync.dma_start(crecip_sb[:], crecip_in[:])
            wpc_sb = constp.tile([128, 4], f32)
            nc.sync.dma_start(wpc_sb[:], wpc_in[:])
            ownsel_sb = constp.tile([128, 4 * L], f16)
            nc.sync.dma_start(ownsel_sb[:], ownsel_in[:])
            perms_sb = constp.tile([L, (L - 1) * L], f16)
            nc.sync.dma_start(perms_sb[:], perms_in[:])
            pmk_sb = constp.tile([L, L - 1], f32)
            nc.sync.dma_start(pmk_sb[:], pmk_in[:])
            ones64_sb = constp.tile([L, 1], f32)
            nc.sync.dma_start(ones64_sb[:], ones64_in[:])
            negdv_sb = constp.tile([128, 1], f32)
            nc.vector.memset(negdv_sb[:], -DELTA_V)
            twodd_sb = constp.tile([L, 1], f32)
            nc.vector.memset(twodd_sb[:], 2.0 * DELTA_D)

            for rep in range(reps):
                # normalized points, resident across both passes
                xs = xsp.tile([128, T, D], f16, tag="xs", name="xs")
                ss_all = normp.tile([128, T], f32, tag="ss", name="ss_all")
                rr_all = normp.tile([128, T], f32, tag="rr", name="rr_all")

                # ---- pass 1: normalize + per-core segment sums ----
                with (
                    tc.tile_pool(name="psum1", bufs=1, space="PSUM") as psum1,
                    tc.tile_pool(name="x1", bufs=3) as xp1,
                    tc.tile_pool(name="oh1", bufs=3) as ohp,
                    tc.tile_pool(name="sqc", bufs=2) as sqcp,
                ):
                    ps_sums = [
                        psum1.tile([128, D], f32, tag=f"sums{c}", name=f"ps_sums{c}")
                        for c in range(4)
                    ]
                    for g in range(T // G):
                        t0 = g * G
                        xb = xp1.tile([128, G, D], f16, tag="x1t")
                        nc.sync.dma_start(
                            xb[:],
                            x_in[t0 * 128:(t0 + G) * 128, :].rearrange(
                                "(g p) d -> p g d", p=128),
                        )
                        ohb = ohp.tile([128, G, NSEG], f16, tag="oh1t")
                        nc.sync.dma_start(
                            ohb[:],
                            oh_in[t0 * 128:(t0 + G) * 128, :].rearrange(
                                "(g p) s -> p g s", p=128),
                        )
                        for j in range(G):
                            t = t0 + j
                            sink = sqcp.tile([128, D], f16, tag="sq_sink")
                            if j % 2 == 0:
                                nc.vector.scalar_tensor_tensor(
                                    sink[:], xb[:, j, :], 1.0, xb[:, j, :],
                                    op0=AluOp.bypass, op1=AluOp.mult,
                                    accum_out=ss_all[:, t:t + 1],
                                )
                            else:
                                nc.scalar.activation(
                                    sink[:], xb[:, j, :], ActFn.Square,
                                    accum_out=ss_all[:, t:t + 1],
                                )
                        sqc = sqcp.tile([128, G], f32, tag="sqc")
                        nc.scalar.activation(
                            sqc[:], ss_all[:, t0:t0 + G], ActFn.Sqrt
                        )
                        nc.vector.tensor_scalar_add(sqc[:], sqc[:], 1e-8)
                        nc.vector.reciprocal(rr_all[:, t0:t0 + G], sqc[:])
                        for j in range(G):
                            t = t0 + j
                            # x_hat = x * rr, stored resident for pass 2
                            nc.vector.tensor_scalar(
                                xs[:, t, :], xb[:, j, :],
                                rr_all[:, t:t + 1], None, op0=AluOp.mult,
                            )
                            for c in range(4):
                                nc.tensor.matmul(
                                    ps_sums[c][:],
                                    ohb[:, j, c * 128:(c + 1) * 128],
                                    xs[:, t, :],
                                    start=(t == 0), stop=(t == T - 1),
                                )

                    ar_in = dram.tile([NSEG, D], f32, tag="ar_in", name="ar_in")
                    for c in range(4):
                        sums_sb = sqcp.tile(
                            [128, D], f32, tag="sums_sb", name="sums_sb"
                        )
                        nc.vector.tensor_copy(sums_sb[:], ps_sums[c][:])
                        nc.sync.dma_start(
                            ar_in[c * 128:(c + 1) * 128, :], sums_sb[:]
                        )

                # ---- single AllReduce of the [512, 256] sum table ----
                ar_out = dram.tile(
                    [NSEG, D], f32, tag="ar_out", name="ar_out",
                    addr_space="Shared" if use_collectives else "Local",
                )
                if use_collectives:
                    nc.gpsimd.collective_compute(
                        "AllReduce", AluOp.add,
                        replica_groups=[list(range(NCORES))],
                        ins=[ar_in.opt()], outs=[ar_out.opt()],
                    )
                else:
                    nc.sync.dma_start(ar_out[:], ar_in[:])

                # ---- centroid table: mu = sums * crecip, plus w column ----
                # layout [128, c, D+1]: partition p = seg-in-block, block c
                mut_sb = mup.tile([128, 4, D + 1], f32, tag="mut", name="mut_sb")
                for c in range(4):
                    nc.sync.dma_start(
                        mut_sb[:, c, 0:D], ar_out[c * 128:(c + 1) * 128, :]
                    )
                mut_h = mup.tile([128, 4, D + 1], f16, tag="muth", name="mut_h")
                for c in range(4):
                    nc.vector.tensor_scalar(
                        mut_h[:, c, 0:D], mut_sb[:, c, 0:D],
                        crecip_sb[:, c:c + 1], None, op0=AluOp.mult,
                    )
                nc.vector.tensor_copy(mut_h[:, :, D], wpc_sb[:])

                # ---- push: own-subbatch pairwise centroid L1 distances ----
                # SPMD-identical device code; the per-core ownsel input picks
                # this core's 64 subbatch rows out of the 4x128 mu table.
                q_sb = mup.tile([L, L - 1], f32, tag="q", name="q_sb")
                mua_h = mup.tile([L, D], f16, tag="muah", name="mua_h")
                with (
                    tc.tile_pool(name="rotps", bufs=2, space="PSUM") as rotpsp,
                    tc.tile_pool(name="pushps", bufs=1, space="PSUM") as pushpsp,
                    tc.tile_pool(name="pdiff", bufs=3) as pdp,
                ):
                    ps_own = rotpsp.tile([L, D], f32, tag="rotps")
                    for c in range(4):
                        nc.tensor.matmul(
                            ps_own[:], ownsel_sb[:, c * L:(c + 1) * L],
                            mut_h[:, c, 0:D],
                            start=(c == 0), stop=(c == 3),
                        )
                    nc.vector.tensor_copy(mua_h[:], ps_own[:])
                    for k in range(1, L):
                        ps_rot = rotpsp.tile([L, D], f32, tag="rotps")
                        nc.tensor.matmul(
                            ps_rot[:], perms_sb[:, (k - 1) * L:k * L], mua_h[:],
                            start=True, stop=True,
                        )
                        pdiff = pdp.tile([L, D], f32, tag="pdiff")
                        nc.vector.tensor_sub(pdiff[:], mua_h[:], ps_rot[:])
                        psink = pdp.tile([L, D], f32, tag="psink")
                        nc.scalar.activation(
                            psink[:], pdiff[:], ActFn.Abs,
                            accum_out=q_sb[:, k - 1:k],
                        )
                    # push = sum(relu(2*dd - q)^2 * pmk) / max(M(M-1),1)
                    relq = pdp.tile([L, L - 1], f32, tag="relq")
                    nc.scalar.activation(
                        relq[:], q_sb[:], ActFn.Relu,
                        bias=twodd_sb[:], scale=-1.0,
                    )
                    vv = pdp.tile([L, L - 1], f32, tag="vv")
                    nc.vector.tensor_mul(vv[:], relq[:], relq[:])
                    nc.vector.tensor_mul(vv[:], vv[:], pmk_sb[:])
                    sink1 = pdp.tile([L, L - 1], f32, tag="sink1")
                    v1 = pdp.tile([L, 1], f32, tag="v1")
                    nc.scalar.activation(
                        sink1[:], vv[:], ActFn.Abs, accum_out=v1[:],
                    )
                    ps_push = pushpsp.tile([1, 1], f32, tag="push")
                    nc.tensor.matmul(
                        ps_push[:], ones64_sb[:], v1[:], start=True, stop=True,
                    )
                    push_sb = pdp.tile([1, 1], f32, tag="push_sb")
                    nc.vector.tensor_copy(push_sb[:], ps_push[:])
                    nc.sync.dma_start(push_out[:], push_sb[:])

                # ---- pass 2: pull term ----
                with (
                    tc.tile_pool(name="oht2", bufs=3) as ohtp,
                    tc.tile_pool(name="ohsb2", bufs=2) as ohsbp,
                    tc.tile_pool(name="mups", bufs=3, space="PSUM") as mupsp,
                    tc.tile_pool(name="pullps", bufs=1, space="PSUM") as pullpsp,
                    tc.tile_pool(name="diff", bufs=3) as diffp,
                    tc.tile_pool(name="sink2", bufs=2) as sink2p,
                    tc.tile_pool(name="small", bufs=4) as smallp,
                ):
                    ps_pull = pullpsp.tile([S, 1], f32, tag="pull", name="ps_pull")
                    for t in range(T):
                        j = t % G
                        if j == 0:
                            ohtb = ohtp.tile([128, G, NSEG], f16, tag="oht")
                            nc.sync.dma_start(
                                ohtb[:],
                                oht_in[t * 128:(t + G) * 128, :].rearrange(
                                    "(g p) s -> p g s", p=128),
                            )
                            ohsbb = ohsbp.tile([128, G, S], f16, tag="ohsb")
                            nc.sync.dma_start(
                                ohsbb[:],
                                ohsb_in[t * 128:(t + G) * 128, :].rearrange(
                                    "(g p) s -> p g s", p=128),
                            )
                        ps_mu = mupsp.tile([128, D + 1], f32, tag="mu")
                        for c in range(4):
                            nc.tensor.matmul(
                                ps_mu[:],
                                ohtb[:, j, c * 128:(c + 1) * 128],
                                mut_h[:, c, :],
                                start=(c == 0), stop=(c == 3),
                            )
                        diff = diffp.tile([128, D], f32, tag="diff")
                        nc.vector.tensor_sub(diff[:], xs[:, t, :], ps_mu[:, 0:D])
                        sink = sink2p.tile([128, D], f32, tag="sink2")
                        d1 = smallp.tile([128, 1], f32, tag="d1")
                        nc.scalar.activation(
                            sink[:], diff[:], ActFn.Abs, accum_out=d1[:]
                        )
                        t1 = smallp.tile([128, 1], f32, tag="t1")
                        nc.scalar.activation(
                            t1[:], d1[:], ActFn.Relu, bias=negdv_sb[:]
                        )
                        t2 = smallp.tile([128, 1], f32, tag="t2")
                        nc.vector.tensor_mul(t2[:], t1[:], t1[:])
                        v = smallp.tile([128, 1], f16, tag="v")
                        nc.vector.tensor_mul(v[:], t2[:], ps_mu[:, D:D + 1])
                        nc.tensor.matmul(
                            ps_pull[:], ohsbb[:, j, :], v[:],
                            start=(t == 0), stop=(t == T - 1),
                        )
                    lpull_sb = smallp.tile([S, 1], f32, tag="lpull_sb")
                    nc.vector.tensor_copy(lpull_sb[:], ps_pull[:])
                    nc.sync.dma_start(lpull_out[:], lpull_sb[:])

    nc.compile()
    return nc


def host_tables(labels: np.ndarray, subbatch: np.ndarray):
    """Everything derivable from the integer inputs alone."""
    seg = (subbatch.astype(np.int64) * L + labels.astype(np.int64)).astype(np.int32)
    counts = np.bincount(seg, minlength=NSEG).astype(np.float64)  # [512]
    present = counts > 0
    M = present.reshape(S, L).sum(axis=1).astype(np.float64)  # [S]
    valid = M > 1.0
    M_per_seg = np.repeat(M, L)
    valid_per_seg = np.repeat(valid, L)
    w = np.where(
        valid_per_seg, 1.0 / (M_per_seg * np.maximum(counts, 1.0)), 0.0
    ).astype(np.float32)
    crecip = (1.0 / np.maximum(counts, 1.0)).astype(np.float32)
    return seg, counts, present, M, valid, w, crecip


def make_in_maps(outputs: np.ndarray, labels: np.ndarray, subbatch: np.ndarray):
    n = outputs.shape[0]
    n_core = n // NCORES
    T = n_core // 128
    seg, counts, present, M, valid, w, crecip = host_tables(labels, subbatch)

    iota512 = np.arange(NSEG, dtype=np.int32)

    # per-(p,c) tables, seg = c*128 + p
    crecip_pc = np.ascontiguousarray(crecip.reshape(4, 128).T)  # [128, 4]
    wpc = np.ascontiguousarray(w.reshape(4, 128).T)  # [128, 4]

    # rotation matrices perm_k[p, a] = 1 iff p == (a + k) % 64, k = 1..63
    a = np.arange(L)
    perms = np.zeros((L, (L - 1) * L), dtype=np.float16)
    for k in range(1, L):
        pm = (a[:, None] == ((a[None, :] + k) % L))
        perms[:, (k - 1) * L:k * L] = pm.astype(np.float16)

    pres_sl = present.reshape(S, L)

    in_maps = []
    for c in range(NCORES):
        sl = slice(c * n_core, (c + 1) * n_core)
        segc = seg[sl]
        sbc = subbatch[sl].astype(np.int64)

        oh = (segc[:, None] == iota512[None, :]).astype(np.float16)
        # transposed layout: oht[t*128 + p, cb*128 + j] = oh[t*128 + j, cb*128 + p]
        oht = np.ascontiguousarray(
            oh.reshape(T, 128, 4, 128).transpose(0, 3, 2, 1).reshape(n_core, NSEG)
        )
        ohsb = (sbc[:, None] == np.arange(S)[None, :]).astype(np.float16)

        # own-subbatch row selector: core c's subbatch segs c*64..c*64+63
        # sit at partitions 64*(c%2).. of block c//2 in the mu table.
        # ownsel[p, cb*64 + a] = 1 iff cb == c//2 and p == 64*(c%2) + a
        ownsel = np.zeros((128, 4 * L), dtype=np.float16)
        blk = c // 2
        off = 64 * (c % 2)
        ownsel[off + a, blk * L + a] = 1.0

        p_own = pres_sl[c]
        mask = (p_own[:, None] & p_own[None, :] & ~np.eye(L, dtype=bool))
        denom = max(M[c] * (M[c] - 1.0), 1.0)
        scale = (1.0 / denom) if valid[c] else 0.0
        # pmk[a, k-1] = mask[a, (a+k)%64] * scale
        pmk = np.zeros((L, L - 1), dtype=np.float32)
        for k in range(1, L):
            pmk[:, k - 1] = mask[a, (a + k) % L] * scale

        in_maps.append({
            "x": np.ascontiguousarray(outputs[sl]).astype(np.float16),
            "oh": oh,
            "oht": oht,
            "ohsb": ohsb,
            "crecip": crecip_pc,
            "wpc": wpc,
            "ownsel": ownsel,
            "perms": perms,
            "pmk": pmk,
            "ones64": np.ones((L, 1), dtype=np.float32),
        })
    return in_maps, (seg, counts, present, M, valid, w, crecip)


def combine(results, tables, n: int):
    """Host combine of the per-core outputs into the scalar loss."""
    total = np.float64(0.0)
    for r in results:
        total += r["lpull"].astype(np.float64).sum()
        total += r["push"].astype(np.float64).sum()
    return np.float32(total / n)


_NC_CACHE: dict = {}


def _get_nc(n_core: int):
    if n_core not in _NC_CACHE:
        _NC_CACHE[n_core] = build_nc(n_core)
    return _NC_CACHE[n_core]


def kernel(outputs, labels, subbatch_indices):
    from concourse.bass_utils import run_bass_kernel_spmd

    outputs = np.asarray(outputs, dtype=np.float32)
    labels = np.asarray(labels, dtype=np.int32)
    subbatch_indices = np.asarray(subbatch_indices, dtype=np.int32)
    n = outputs.shape[0]
    n_core = n // NCORES

    nc = _get_nc(n_core)
    in_maps, tables = make_in_maps(outputs, labels, subbatch_indices)
    res = run_bass_kernel_spmd(nc, in_maps, list(range(NCORES)))
    return combine(res.results, tables, n)
